# revision 9
# baseline (speedup 1.0000x reference)
"""2-layer GAT (GATConv x2, mean-over-heads) on 8 Trainium2 NeuronCores.

Strategy (edge/segment parallel):
  - Host: add self loops, sort edges by destination, partition destination
    nodes into 8 contiguous equal shards (random graph => edge counts are
    statistically balanced).  Each core owns all edges of its dst shard, so
    segment softmax/sums never cross cores (no partial-sum all-reduce).
  - Device phase A (per layer, replicated): G = X @ Wcat where
    Wcat = [W.T | W.T@A_src | W.T@A_dst]  ->  G[n] = [xh(512) | a_s(4) | a_d(4)]
  - Device phase B (per layer, edge-parallel): for each 128-dst-node tile,
    process edges in chunks of 128 (one edge per partition):
      gather G[src[e]] (indirect DMA), gather a_d = G[dst[e], 516:520],
      alpha = lrelu(a_s + a_d), ex = exp(alpha)   (softmax max-shift is
      skipped: alpha is O(1) and softmax is shift-invariant),
      mask[e,p] = (dst_local[e] == p)  (iota compare),
      out[p, h*C:..] += (mask * ex_h).T @ xh_gathered_h   (PE matmul)
      den[p, h] += mask.T @ ex                            (PE matmul)
    then h = sum_h out_h / (H*(den_h + eps)) + b.
  - AllGather h shards between layers (layer-2 needs all-node features).
  - Core output = its 1/8 shard of rows; host concatenates.
"""

import numpy as np

P = 128
NCORES = 8

_CACHE = {}
DEBUG = False


def _host_prep(x, edge_index, W1, att_src1, att_dst1, b1, W2, att_src2, att_dst2, b2):
    N, IN_F = x.shape
    HEADS, HID = att_src1.shape
    OUT_F = att_src2.shape[1]
    E2 = edge_index.shape[1]

    # --- edges: self loops + sort by dst ---
    src = np.concatenate([edge_index[0], np.arange(N, dtype=np.int64)])
    dst = np.concatenate([edge_index[1], np.arange(N, dtype=np.int64)])
    order = np.argsort(dst, kind="stable")
    srcs = src[order].astype(np.int32)
    dsts = dst[order].astype(np.int32)
    E = srcs.shape[0]

    NP_ = -(-N // (NCORES * P)) * (NCORES * P)   # padded node count
    NT = NP_ // P                                # node tiles (global)
    TPC = NT // NCORES                           # tiles per core

    tile_of_edge = dsts // P
    counts = np.bincount(tile_of_edge, minlength=NT)
    K = max(1, int(np.max(-(-counts // P))))     # chunks per tile (uniform)

    bounds = np.concatenate([[0], np.cumsum(counts)])
    within = np.arange(E) - bounds[tile_of_edge]

    SRCp = np.zeros((NT, K * P), np.int32)
    DSTGp = np.zeros((NT, K * P), np.int32)
    DSTLp = np.full((NT, K * P), -1.0, np.float32)
    SRCp[tile_of_edge, within] = srcs
    DSTGp[tile_of_edge, within] = dsts
    DSTLp[tile_of_edge, within] = (dsts - tile_of_edge * P).astype(np.float32)

    def per_core(arr):
        # [NT, K*P] -> list of [P, TPC*K] (column ci = chunk ci's 128 edges)
        out = []
        for i in range(NCORES):
            a = arr[i * TPC:(i + 1) * TPC].reshape(TPC * K, P)
            out.append(np.ascontiguousarray(a.T))
        return out

    SRC_cores = per_core(SRCp)
    DSTG_cores = per_core(DSTGp)
    DSTL_cores = per_core(DSTLp)

    # --- weights: Wcat = [W.T | W.T@A_src | W.T@A_dst] ---
    def wcat(W, att_s, att_d, in_f, ch):
        As = np.zeros((HEADS * ch, HEADS), np.float32)
        Ad = np.zeros((HEADS * ch, HEADS), np.float32)
        for h in range(HEADS):
            As[h * ch:(h + 1) * ch, h] = att_s[h]
            Ad[h * ch:(h + 1) * ch, h] = att_d[h]
        WT = W.T.astype(np.float32)                       # [in_f, HEADS*ch]
        return np.concatenate([WT, WT @ As, WT @ Ad], axis=1)  # [in_f, HEADS*ch+8]

    W1cat = np.ascontiguousarray(wcat(W1, att_src1, att_dst1, IN_F, HID))
    W2cat = np.ascontiguousarray(wcat(W2, att_src2, att_dst2, HID, OUT_F))

    xT = np.zeros((IN_F, NP_), np.float32)
    xT[:, :N] = x.T

    IOTA = np.broadcast_to(np.arange(P, dtype=np.float32), (P, P)).copy()
    IDENT = np.eye(P, dtype=np.float32)
    B1bc = np.broadcast_to(b1.astype(np.float32), (P, HID)).copy()
    B2bc = np.broadcast_to(b2.astype(np.float32), (P, OUT_F)).copy()

    shapes = dict(N=N, IN_F=IN_F, HEADS=HEADS, HID=HID, OUT_F=OUT_F,
                  NP=NP_, NT=NT, TPC=TPC, K=K)
    shared = dict(xT=xT, W1cat=W1cat, W2cat=W2cat, IOTA=IOTA, IDENT=IDENT,
                  B1bc=B1bc, B2bc=B2bc)
    percore = [dict(SRC=SRC_cores[i], DSTG=DSTG_cores[i], DSTL=DSTL_cores[i])
               for i in range(NCORES)]
    return shapes, shared, percore


def _build(s):
    import concourse.bass as bass
    import concourse.mybir as mybir
    import concourse.tile as tile
    from concourse import bacc

    f32 = mybir.dt.float32
    i32 = mybir.dt.int32
    HEADS, HID, OUT_F, IN_F = s["HEADS"], s["HID"], s["OUT_F"], s["IN_F"]
    NP_, NT, TPC, K = s["NP"], s["NT"], s["TPC"], s["K"]
    GW1 = HEADS * HID + 2 * HEADS      # 520
    GW2 = HEADS * OUT_F + 2 * HEADS
    NCH = TPC * K
    KC1 = IN_F // P
    AluOp = mybir.AluOpType
    Act = mybir.ActivationFunctionType

    nc = bacc.Bacc("TRN2", target_bir_lowering=False, debug=False,
                   num_devices=NCORES)

    t_xT = nc.dram_tensor("xT", [IN_F, NP_], f32, kind="ExternalInput")
    t_w1 = nc.dram_tensor("W1cat", [IN_F, GW1], f32, kind="ExternalInput")
    t_w2 = nc.dram_tensor("W2cat", [HID, GW2], f32, kind="ExternalInput")
    t_iota = nc.dram_tensor("IOTA", [P, P], f32, kind="ExternalInput")
    t_ident = nc.dram_tensor("IDENT", [P, P], f32, kind="ExternalInput")
    t_b1 = nc.dram_tensor("B1bc", [P, HID], f32, kind="ExternalInput")
    t_b2 = nc.dram_tensor("B2bc", [P, OUT_F], f32, kind="ExternalInput")
    t_src = nc.dram_tensor("SRC", [P, NCH], i32, kind="ExternalInput")
    t_dstg = nc.dram_tensor("DSTG", [P, NCH], i32, kind="ExternalInput")
    t_dstl = nc.dram_tensor("DSTL", [P, NCH], f32, kind="ExternalInput")
    t_out = nc.dram_tensor("out", [TPC * P, OUT_F], f32, kind="ExternalOutput")
    t_gdbg = t_hdbg = None
    if DEBUG:
        t_gdbg = nc.dram_tensor("gdbg", [2 * P, GW1], f32, kind="ExternalOutput")
        t_hdbg = nc.dram_tensor("hdbg", [TPC * P, HID], f32, kind="ExternalOutput")
        t_cdbg = nc.dram_tensor("cdbg", [P, 152], f32, kind="ExternalOutput")
        t_cdbg2 = nc.dram_tensor("cdbg2", [P, 264], f32, kind="ExternalOutput")

    with tile.TileContext(nc) as tc:
        with tc.tile_pool(name="const", bufs=1) as constp, \
             tc.tile_pool(name="dram", bufs=1, space="DRAM") as dramp, \
             tc.tile_pool(name="lhs", bufs=3) as lhsp, \
             tc.tile_pool(name="stage", bufs=3) as stagep, \
             tc.tile_pool(name="gat", bufs=6) as gatp, \
             tc.tile_pool(name="msk", bufs=6) as mskp, \
             tc.tile_pool(name="small", bufs=6) as smallp, \
             tc.tile_pool(name="accs", bufs=3) as accp:

            G1 = dramp.tile([NP_, GW1], f32, name="G1")
            G2 = dramp.tile([NP_, GW2], f32, name="G2")
            h_sh = dramp.tile([TPC * P, HID], f32, name="h_sh")
            h_full = dramp.tile([NP_, HID], f32, name="h_full",
                                addr_space="Shared")

            iota_sb = constp.tile([P, P], f32, name="iota_sb")
            nc.sync.dma_start(out=iota_sb[:], in_=t_iota[:, :])
            ident_sb = constp.tile([P, P], f32, name="ident_sb")
            nc.sync.dma_start(out=ident_sb[:], in_=t_ident[:, :])
            b1_sb = constp.tile([P, HID], f32, name="b1_sb")
            nc.sync.dma_start(out=b1_sb[:], in_=t_b1[:, :])
            b2_sb = constp.tile([P, OUT_F], f32, name="b2_sb")
            nc.sync.dma_start(out=b2_sb[:], in_=t_b2[:, :])
            w1_sb = []
            for k in range(KC1):
                w1k = constp.tile([P, GW1], f32, name=f"w1_sb{k}")
                nc.sync.dma_start(out=w1k[:], in_=t_w1[k * P:(k + 1) * P, :])
                w1_sb.append(w1k)
            w2_sb = constp.tile([P, GW2], f32, name="w2_sb")
            nc.sync.dma_start(out=w2_sb[:], in_=t_w2[:, :])
            src_sb = constp.tile([P, NCH], i32, name="src_sb")
            nc.sync.dma_start(out=src_sb[:], in_=t_src[:, :])
            dstg_sb = constp.tile([P, NCH], i32, name="dstg_sb")
            nc.sync.dma_start(out=dstg_sb[:], in_=t_dstg[:, :])
            dstl_sb = constp.tile([P, NCH], f32, name="dstl_sb")
            nc.sync.dma_start(out=dstl_sb[:], in_=t_dstl[:, :])

            NH = HEADS * HID  # 512

            def phase_a_l1():
              with tc.tile_pool(name="psA", bufs=2, space="PSUM") as psA:
                for t in range(NT):
                    ps = psA.tile([P, NH], f32, name="aps")
                    ps2 = psA.tile([P, 2 * HEADS], f32, name="aps2")
                    for k in range(KC1):
                        lx = lhsp.tile([P, P], f32, name="lx")
                        nc.sync.dma_start(
                            out=lx[:],
                            in_=t_xT[k * P:(k + 1) * P, t * P:(t + 1) * P])
                        nc.tensor.matmul(ps[:, :], lhsT=lx[:],
                                         rhs=w1_sb[k][:, 0:NH],
                                         start=(k == 0), stop=(k == KC1 - 1))
                        nc.tensor.matmul(ps2[:, :], lhsT=lx[:],
                                         rhs=w1_sb[k][:, NH:GW1],
                                         start=(k == 0), stop=(k == KC1 - 1))
                    st = stagep.tile([P, GW1], f32, name="st")
                    nc.scalar.copy(st[:, 0:NH], ps[:, :])
                    nc.vector.tensor_copy(st[:, NH:GW1], ps2[:, :])
                    nc.sync.dma_start(out=G1[t * P:(t + 1) * P, :], in_=st[:])

            def phase_a_l2():
              with tc.tile_pool(name="psA2", bufs=2, space="PSUM") as psA:
                for t in range(NT):
                    hh = lhsp.tile([P, HID], f32, name="hh")
                    nc.sync.dma_start(out=hh[:],
                                      in_=h_full[t * P:(t + 1) * P, :])
                    tp = psA.tile([P, P], f32, name="tps")
                    nc.tensor.transpose(tp[:], hh[:], ident_sb[:])
                    hT = lhsp.tile([P, P], f32, name="hT")
                    nc.scalar.copy(hT[:], tp[:])
                    ps = psA.tile([P, NH], f32, name="aps")
                    ps2 = psA.tile([P, 2 * HEADS], f32, name="aps2")
                    nc.tensor.matmul(ps[:, :], lhsT=hT[:], rhs=w2_sb[:, 0:NH],
                                     start=True, stop=True)
                    nc.tensor.matmul(ps2[:, :], lhsT=hT[:], rhs=w2_sb[:, NH:GW2],
                                     start=True, stop=True)
                    st = stagep.tile([P, GW2], f32, name="st")
                    nc.scalar.copy(st[:, 0:NH], ps[:, :])
                    nc.vector.tensor_copy(st[:, NH:GW2], ps2[:, :])
                    nc.sync.dma_start(out=G2[t * P:(t + 1) * P, :], in_=st[:])

            def phase_b(G, GW, bbc_sb, out_dram):
              with tc.tile_pool(name="psB", bufs=2, space="PSUM") as psB:
                for t in range(TPC):
                    out_ps = psB.tile([P, NH], f32, name="outps")
                    den_ps = psB.tile([P, HEADS], f32, name="denps")
                    for c in range(K):
                        ci = t * K + c
                        first, last = (c == 0), (c == K - 1)
                        g = gatp.tile([P, GW], f32, name="g")
                        nc.gpsimd.indirect_dma_start(
                            out=g[:], out_offset=None, in_=G[:, :],
                            in_offset=bass.IndirectOffsetOnAxis(
                                ap=src_sb[:, ci:ci + 1], axis=0))
                        adg = smallp.tile([P, HEADS], f32, name="adg")
                        nc.gpsimd.indirect_dma_start(
                            out=adg[:], out_offset=None, in_=G[:, :],
                            in_offset=bass.IndirectOffsetOnAxis(
                                ap=dstg_sb[:, ci:ci + 1], axis=0),
                            element_offset=NH + HEADS)
                        mask = mskp.tile([P, P], f32, name="mask")
                        nc.vector.tensor_scalar(
                            out=mask[:], in0=iota_sb[:],
                            scalar1=dstl_sb[:, ci:ci + 1], scalar2=None,
                            op0=AluOp.is_equal)
                        # msgex[:, 0:NH] = ex_h * xh_gathered ; [:, NH:NH+4] = ex
                        msgex = mskp.tile([P, NH + HEADS], f32, name="msgex")
                        ex = msgex[:, NH:NH + HEADS]
                        nc.vector.tensor_tensor(out=ex, in0=g[:, NH:NH + HEADS],
                                                in1=adg[:], op=AluOp.add)
                        t2 = smallp.tile([P, HEADS], f32, name="t2")
                        nc.vector.tensor_scalar_mul(t2[:], ex, 0.2)
                        nc.vector.tensor_tensor(out=ex, in0=ex, in1=t2[:],
                                                op=AluOp.max)
                        nc.scalar.activation(out=ex, in_=ex, func=Act.Exp)
                        for h in range(HEADS):
                            nc.vector.tensor_scalar_mul(
                                msgex[:, h * HID:(h + 1) * HID],
                                g[:, h * HID:(h + 1) * HID],
                                msgex[:, NH + h:NH + h + 1])
                        nc.tensor.matmul(out_ps[:, :], lhsT=mask[:],
                                         rhs=msgex[:, 0:NH],
                                         start=first, stop=last)
                        nc.tensor.matmul(den_ps[:, :], lhsT=mask[:],
                                         rhs=msgex[:, NH:NH + HEADS],
                                         start=first, stop=last)
                    den4 = smallp.tile([P, HEADS], f32, name="den4")
                    nc.vector.tensor_scalar(
                        out=den4[:], in0=den_ps[:], scalar1=float(HEADS),
                        scalar2=float(HEADS) * 1e-16, op0=AluOp.mult,
                        op1=AluOp.add)
                    rec = smallp.tile([P, HEADS], f32, name="rec")
                    nc.vector.reciprocal(rec[:], den4[:])
                    if DEBUG and G is G1 and t == 0:
                        cst2 = stagep.tile([P, 264], f32, name="cdbg2_st")
                        nc.vector.tensor_copy(cst2[:, 0:4], den_ps[:])
                        nc.vector.tensor_copy(cst2[:, 4:8], den4[:])
                        nc.vector.tensor_copy(cst2[:, 8:136], out_ps[:, 0:128])
                        nc.vector.tensor_copy(cst2[:, 136:264], out_ps[:, 384:512])
                        nc.sync.dma_start(out=t_cdbg2[:, :], in_=cst2[:])
                    acc = accp.tile([P, HID], f32, name="acc")
                    tmp = accp.tile([P, HID], f32, name="tmpacc")
                    nc.vector.tensor_scalar_mul(acc[:], out_ps[:, 0:HID],
                                                rec[:, 0:1])
                    for h in range(1, HEADS):
                        nc.vector.tensor_scalar_mul(
                            tmp[:], out_ps[:, h * HID:(h + 1) * HID],
                            rec[:, h:h + 1])
                        nc.vector.tensor_tensor(out=acc[:], in0=acc[:],
                                                in1=tmp[:], op=AluOp.add)
                    nc.vector.tensor_tensor(out=acc[:], in0=acc[:], in1=bbc_sb[:],
                                            op=AluOp.add)
                    nc.sync.dma_start(out=out_dram[t * P:(t + 1) * P, :],
                                      in_=acc[:])

            phase_a_l1()
            phase_b(G1, GW1, b1_sb, h_sh)
            if DEBUG:
                nc.sync.dma_start(out=t_gdbg[:, :], in_=G1[0:2 * P, :])
                nc.sync.dma_start(out=t_hdbg[:, :], in_=h_sh[:, :])
            nc.gpsimd.collective_compute(
                "AllGather", AluOp.bypass,
                replica_groups=[list(range(NCORES))],
                ins=[h_sh[:].opt()], outs=[h_full[:].opt()])
            phase_a_l2()
            phase_b(G2, GW2, b2_sb, t_out[:, :])

    nc.compile()
    return nc


def _get_nc(s):
    key = tuple(sorted(s.items()))
    if key not in _CACHE:
        _CACHE[key] = _build(s)
    return _CACHE[key]


def kernel(**inputs):
    from concourse import bass_utils

    x = np.asarray(inputs["x"], dtype=np.float32)
    edge_index = np.asarray(inputs["edge_index"])
    args = (x, edge_index,
            np.asarray(inputs["W1"], np.float32),
            np.asarray(inputs["att_src1"], np.float32),
            np.asarray(inputs["att_dst1"], np.float32),
            np.asarray(inputs["b1"], np.float32),
            np.asarray(inputs["W2"], np.float32),
            np.asarray(inputs["att_src2"], np.float32),
            np.asarray(inputs["att_dst2"], np.float32),
            np.asarray(inputs["b2"], np.float32))
    shapes, shared, percore = _host_prep(*args)
    nc = _get_nc(shapes)

    in_maps = []
    for i in range(NCORES):
        m = {"xT": shared["xT"], "W1cat": shared["W1cat"],
             "W2cat": shared["W2cat"], "IOTA": shared["IOTA"],
             "IDENT": shared["IDENT"], "B1bc": shared["B1bc"],
             "B2bc": shared["B2bc"], "SRC": percore[i]["SRC"],
             "DSTG": percore[i]["DSTG"], "DSTL": percore[i]["DSTL"]}
        in_maps.append(m)

    res = bass_utils.run_bass_kernel_spmd(nc, in_maps,
                                          core_ids=list(range(NCORES)))
    out = np.concatenate([res.results[i]["out"] for i in range(NCORES)],
                         axis=0)[:shapes["N"]]
    return np.ascontiguousarray(out, dtype=np.float32)


# revision 11
# speedup vs baseline: 1.3726x; 1.3726x over previous
"""2-layer GAT (GATConv x2, mean-over-heads) on 8 Trainium2 NeuronCores.

Strategy (edge/segment parallel):
  - Host: add self loops, sort edges by destination, partition destination
    nodes into 8 contiguous equal shards (random graph => edge counts are
    statistically balanced).  Each core owns all edges of its dst shard, so
    segment softmax/sums never cross cores (no partial-sum all-reduce).
  - Device phase A (per layer, replicated): G = X @ Wcat where
    Wcat = [W.T | W.T@A_src | W.T@A_dst]  ->  G[n] = [xh(512) | a_s(4) | a_d(4)]
  - Device phase B (per layer, edge-parallel): for each 128-dst-node tile,
    process edges in chunks of 128 (one edge per partition):
      gather G[src[e]] (indirect DMA), gather a_d = G[dst[e], 516:520],
      alpha = lrelu(a_s + a_d), ex = exp(alpha)   (softmax max-shift is
      skipped: alpha is O(1) and softmax is shift-invariant),
      mask[e,p] = (dst_local[e] == p)  (iota compare),
      out[p, h*C:..] += (mask * ex_h).T @ xh_gathered_h   (PE matmul)
      den[p, h] += mask.T @ ex                            (PE matmul)
    then h = sum_h out_h / (H*(den_h + eps)) + b.
  - AllGather h shards between layers (layer-2 needs all-node features).
  - Core output = its 1/8 shard of rows; host concatenates.
"""

import numpy as np

P = 128
NCORES = 8

_CACHE = {}
DEBUG = False


def _host_prep(x, edge_index, W1, att_src1, att_dst1, b1, W2, att_src2, att_dst2, b2):
    N, IN_F = x.shape
    HEADS, HID = att_src1.shape
    OUT_F = att_src2.shape[1]
    E2 = edge_index.shape[1]

    # --- edges: self loops + sort by dst ---
    src = np.concatenate([edge_index[0], np.arange(N, dtype=np.int64)])
    dst = np.concatenate([edge_index[1], np.arange(N, dtype=np.int64)])
    order = np.argsort(dst, kind="stable")
    srcs = src[order].astype(np.int32)
    dsts = dst[order].astype(np.int32)
    E = srcs.shape[0]

    NP_ = -(-N // (NCORES * P)) * (NCORES * P)   # padded node count
    NT = NP_ // P                                # node tiles (global)
    TPC = NT // NCORES                           # tiles per core

    tile_of_edge = dsts // P
    counts = np.bincount(tile_of_edge, minlength=NT)
    K = max(1, int(np.max(-(-counts // P))))     # chunks per tile (uniform)

    bounds = np.concatenate([[0], np.cumsum(counts)])
    within = np.arange(E) - bounds[tile_of_edge]

    SRCp = np.zeros((NT, K * P), np.int32)
    DSTGp = np.zeros((NT, K * P), np.int32)
    DSTLp = np.full((NT, K * P), -1.0, np.float32)
    SRCp[tile_of_edge, within] = srcs
    DSTGp[tile_of_edge, within] = dsts
    DSTLp[tile_of_edge, within] = (dsts - tile_of_edge * P).astype(np.float32)

    def per_core(arr):
        # [NT, K*P] -> list of [P, TPC*K] (column ci = chunk ci's 128 edges)
        out = []
        for i in range(NCORES):
            a = arr[i * TPC:(i + 1) * TPC].reshape(TPC * K, P)
            out.append(np.ascontiguousarray(a.T))
        return out

    SRC_cores = per_core(SRCp)
    DSTL_cores = per_core(DSTLp)
    TIX_cores = [
        (i * TPC * P + np.arange(TPC * P, dtype=np.int32))
        .reshape(TPC, P).T.copy()
        for i in range(NCORES)
    ]

    # --- weights: Wcat = [W.T | W.T@A_src | W.T@A_dst] ---
    def wcat(W, att_s, att_d, in_f, ch):
        As = np.zeros((HEADS * ch, HEADS), np.float32)
        Ad = np.zeros((HEADS * ch, HEADS), np.float32)
        for h in range(HEADS):
            As[h * ch:(h + 1) * ch, h] = att_s[h]
            Ad[h * ch:(h + 1) * ch, h] = att_d[h]
        WT = W.T.astype(np.float32)                       # [in_f, HEADS*ch]
        return np.concatenate([WT, WT @ As, WT @ Ad], axis=1)  # [in_f, HEADS*ch+8]

    W1cat = np.ascontiguousarray(wcat(W1, att_src1, att_dst1, IN_F, HID))
    W2cat = np.ascontiguousarray(wcat(W2, att_src2, att_dst2, HID, OUT_F))

    xT = np.zeros((IN_F, NP_), np.float32)
    xT[:, :N] = x.T

    IOTA = np.broadcast_to(np.arange(P, dtype=np.float32), (P, P)).copy()
    IDENT = np.eye(P, dtype=np.float32)
    B1bc = np.broadcast_to(b1.astype(np.float32), (P, HID)).copy()
    B2bc = np.broadcast_to(b2.astype(np.float32), (P, OUT_F)).copy()

    shapes = dict(N=N, IN_F=IN_F, HEADS=HEADS, HID=HID, OUT_F=OUT_F,
                  NP=NP_, NT=NT, TPC=TPC, K=K)
    shared = dict(xT=xT, W1cat=W1cat, W2cat=W2cat, IOTA=IOTA, IDENT=IDENT,
                  B1bc=B1bc, B2bc=B2bc)
    percore = [dict(SRC=SRC_cores[i], TILEIDX=TIX_cores[i],
                    DSTL=DSTL_cores[i])
               for i in range(NCORES)]
    return shapes, shared, percore


def _build(s):
    import concourse.bass as bass
    import concourse.mybir as mybir
    import concourse.tile as tile
    from concourse import bacc

    f32 = mybir.dt.float32
    i32 = mybir.dt.int32
    HEADS, HID, OUT_F, IN_F = s["HEADS"], s["HID"], s["OUT_F"], s["IN_F"]
    NP_, NT, TPC, K = s["NP"], s["NT"], s["TPC"], s["K"]
    GW1 = HEADS * HID + 2 * HEADS      # 520
    GW2 = HEADS * OUT_F + 2 * HEADS
    NCH = TPC * K
    KC1 = IN_F // P
    AluOp = mybir.AluOpType
    Act = mybir.ActivationFunctionType

    nc = bacc.Bacc("TRN2", target_bir_lowering=False, debug=False,
                   num_devices=NCORES)

    t_xT = nc.dram_tensor("xT", [IN_F, NP_], f32, kind="ExternalInput")
    t_w1 = nc.dram_tensor("W1cat", [IN_F, GW1], f32, kind="ExternalInput")
    t_w2 = nc.dram_tensor("W2cat", [HID, GW2], f32, kind="ExternalInput")
    t_iota = nc.dram_tensor("IOTA", [P, P], f32, kind="ExternalInput")
    t_ident = nc.dram_tensor("IDENT", [P, P], f32, kind="ExternalInput")
    t_b1 = nc.dram_tensor("B1bc", [P, HID], f32, kind="ExternalInput")
    t_b2 = nc.dram_tensor("B2bc", [P, OUT_F], f32, kind="ExternalInput")
    t_src = nc.dram_tensor("SRC", [P, NCH], i32, kind="ExternalInput")
    t_tix = nc.dram_tensor("TILEIDX", [P, TPC], i32, kind="ExternalInput")
    t_dstl = nc.dram_tensor("DSTL", [P, NCH], f32, kind="ExternalInput")
    t_out = nc.dram_tensor("out", [TPC * P, OUT_F], f32, kind="ExternalOutput")
    t_gdbg = t_hdbg = None
    if DEBUG:
        t_gdbg = nc.dram_tensor("gdbg", [2 * P, GW1], f32, kind="ExternalOutput")
        t_hdbg = nc.dram_tensor("hdbg", [TPC * P, HID], f32, kind="ExternalOutput")
        t_cdbg = nc.dram_tensor("cdbg", [P, 152], f32, kind="ExternalOutput")
        t_cdbg2 = nc.dram_tensor("cdbg2", [P, 264], f32, kind="ExternalOutput")

    with tile.TileContext(nc) as tc:
        with tc.tile_pool(name="const", bufs=1) as constp, \
             tc.tile_pool(name="dram", bufs=1, space="DRAM") as dramp, \
             tc.tile_pool(name="lhs", bufs=3) as lhsp, \
             tc.tile_pool(name="stage", bufs=3) as stagep, \
             tc.tile_pool(name="gat", bufs=6) as gatp, \
             tc.tile_pool(name="msk", bufs=6) as mskp, \
             tc.tile_pool(name="small", bufs=6) as smallp, \
             tc.tile_pool(name="accs", bufs=3) as accp:

            G1 = dramp.tile([NP_, GW1], f32, name="G1")
            G2 = dramp.tile([NP_, GW2], f32, name="G2")
            h_sh = dramp.tile([TPC * P, HID], f32, name="h_sh")
            h_full = dramp.tile([NP_, HID], f32, name="h_full",
                                addr_space="Shared")

            iota_sb = constp.tile([P, P], f32, name="iota_sb")
            nc.sync.dma_start(out=iota_sb[:], in_=t_iota[:, :])
            ident_sb = constp.tile([P, P], f32, name="ident_sb")
            nc.sync.dma_start(out=ident_sb[:], in_=t_ident[:, :])
            b1_sb = constp.tile([P, HID], f32, name="b1_sb")
            nc.sync.dma_start(out=b1_sb[:], in_=t_b1[:, :])
            b2_sb = constp.tile([P, OUT_F], f32, name="b2_sb")
            nc.sync.dma_start(out=b2_sb[:], in_=t_b2[:, :])
            w1_sb = []
            for k in range(KC1):
                w1k = constp.tile([P, GW1], f32, name=f"w1_sb{k}")
                nc.sync.dma_start(out=w1k[:], in_=t_w1[k * P:(k + 1) * P, :])
                w1_sb.append(w1k)
            w2_sb = constp.tile([P, GW2], f32, name="w2_sb")
            nc.sync.dma_start(out=w2_sb[:], in_=t_w2[:, :])
            src_sb = constp.tile([P, NCH], i32, name="src_sb")
            nc.sync.dma_start(out=src_sb[:], in_=t_src[:, :])
            tix_sb = constp.tile([P, TPC], i32, name="tix_sb")
            nc.sync.dma_start(out=tix_sb[:], in_=t_tix[:, :])
            dstl_sb = constp.tile([P, NCH], f32, name="dstl_sb")
            nc.sync.dma_start(out=dstl_sb[:], in_=t_dstl[:, :])

            NH = HEADS * HID  # 512

            def phase_a_l1():
              with tc.tile_pool(name="psA", bufs=2, space="PSUM") as psA:
                for t in range(NT):
                    ps = psA.tile([P, NH], f32, name="aps")
                    ps2 = psA.tile([P, 2 * HEADS], f32, name="aps2")
                    for k in range(KC1):
                        lx = lhsp.tile([P, P], f32, name="lx")
                        nc.sync.dma_start(
                            out=lx[:],
                            in_=t_xT[k * P:(k + 1) * P, t * P:(t + 1) * P])
                        nc.tensor.matmul(ps[:, :], lhsT=lx[:],
                                         rhs=w1_sb[k][:, 0:NH],
                                         start=(k == 0), stop=(k == KC1 - 1))
                        nc.tensor.matmul(ps2[:, :], lhsT=lx[:],
                                         rhs=w1_sb[k][:, NH:GW1],
                                         start=(k == 0), stop=(k == KC1 - 1))
                    st = stagep.tile([P, GW1], f32, name="st")
                    nc.scalar.copy(st[:, 0:NH], ps[:, :])
                    nc.vector.tensor_copy(st[:, NH:GW1], ps2[:, :])
                    nc.sync.dma_start(out=G1[t * P:(t + 1) * P, :], in_=st[:])

            def phase_a_l2():
              with tc.tile_pool(name="psA2", bufs=2, space="PSUM") as psA:
                for t in range(NT):
                    hh = lhsp.tile([P, HID], f32, name="hh")
                    nc.sync.dma_start(out=hh[:],
                                      in_=h_full[t * P:(t + 1) * P, :])
                    tp = psA.tile([P, P], f32, name="tps")
                    nc.tensor.transpose(tp[:], hh[:], ident_sb[:])
                    hT = lhsp.tile([P, P], f32, name="hT")
                    nc.scalar.copy(hT[:], tp[:])
                    ps = psA.tile([P, NH], f32, name="aps")
                    ps2 = psA.tile([P, 2 * HEADS], f32, name="aps2")
                    nc.tensor.matmul(ps[:, :], lhsT=hT[:], rhs=w2_sb[:, 0:NH],
                                     start=True, stop=True)
                    nc.tensor.matmul(ps2[:, :], lhsT=hT[:], rhs=w2_sb[:, NH:GW2],
                                     start=True, stop=True)
                    st = stagep.tile([P, GW2], f32, name="st")
                    nc.scalar.copy(st[:, 0:NH], ps[:, :])
                    nc.vector.tensor_copy(st[:, NH:GW2], ps2[:, :])
                    nc.sync.dma_start(out=G2[t * P:(t + 1) * P, :], in_=st[:])

            def phase_b(G, GW, bbc_sb, out_dram):
              with tc.tile_pool(name="psB", bufs=2, space="PSUM") as psB:
                for t in range(TPC):
                    out_ps = psB.tile([P, NH], f32, name="outps")
                    den_ps = psB.tile([P, HEADS], f32, name="denps")
                    ad_tile = smallp.tile([P, HEADS], f32, name="ad_tile")
                    nc.gpsimd.indirect_dma_start(
                        out=ad_tile[:], out_offset=None, in_=G[:, :],
                        in_offset=bass.IndirectOffsetOnAxis(
                            ap=tix_sb[:, t:t + 1], axis=0),
                        element_offset=NH + HEADS)
                    for c in range(K):
                        ci = t * K + c
                        first, last = (c == 0), (c == K - 1)
                        g = gatp.tile([P, GW], f32, name="g")
                        nc.gpsimd.indirect_dma_start(
                            out=g[:], out_offset=None, in_=G[:, :],
                            in_offset=bass.IndirectOffsetOnAxis(
                                ap=src_sb[:, ci:ci + 1], axis=0))
                        mask = mskp.tile([P, P], f32, name="mask")
                        nc.vector.tensor_scalar(
                            out=mask[:], in0=iota_sb[:],
                            scalar1=dstl_sb[:, ci:ci + 1], scalar2=None,
                            op0=AluOp.is_equal)
                        mtp = psB.tile([P, P], f32, name="mtp")
                        nc.tensor.transpose(mtp[:], mask[:], ident_sb[:])
                        maskT = mskp.tile([P, P], f32, name="maskT")
                        nc.vector.tensor_copy(maskT[:], mtp[:])
                        adpe = psB.tile([P, HEADS], f32, name="adpe")
                        nc.tensor.matmul(adpe[:, :], lhsT=maskT[:],
                                         rhs=ad_tile[:], start=True, stop=True)
                        # msgex[:, 0:NH] = ex_h * xh_gathered ; [:, NH:NH+4] = ex
                        msgex = mskp.tile([P, NH + HEADS], f32, name="msgex")
                        ex = msgex[:, NH:NH + HEADS]
                        nc.vector.tensor_tensor(out=ex, in0=g[:, NH:NH + HEADS],
                                                in1=adpe[:], op=AluOp.add)
                        t2 = smallp.tile([P, HEADS], f32, name="t2")
                        nc.vector.tensor_scalar_mul(t2[:], ex, 0.2)
                        nc.vector.tensor_tensor(out=ex, in0=ex, in1=t2[:],
                                                op=AluOp.max)
                        nc.scalar.activation(out=ex, in_=ex, func=Act.Exp)
                        for h in range(HEADS):
                            nc.vector.tensor_scalar_mul(
                                msgex[:, h * HID:(h + 1) * HID],
                                g[:, h * HID:(h + 1) * HID],
                                msgex[:, NH + h:NH + h + 1])
                        nc.tensor.matmul(out_ps[:, :], lhsT=mask[:],
                                         rhs=msgex[:, 0:NH],
                                         start=first, stop=last)
                        nc.tensor.matmul(den_ps[:, :], lhsT=mask[:],
                                         rhs=msgex[:, NH:NH + HEADS],
                                         start=first, stop=last)
                    den4 = smallp.tile([P, HEADS], f32, name="den4")
                    nc.vector.tensor_scalar(
                        out=den4[:], in0=den_ps[:], scalar1=float(HEADS),
                        scalar2=float(HEADS) * 1e-16, op0=AluOp.mult,
                        op1=AluOp.add)
                    rec = smallp.tile([P, HEADS], f32, name="rec")
                    nc.vector.reciprocal(rec[:], den4[:])
                    if DEBUG and G is G1 and t == 0:
                        cst2 = stagep.tile([P, 264], f32, name="cdbg2_st")
                        nc.vector.tensor_copy(cst2[:, 0:4], den_ps[:])
                        nc.vector.tensor_copy(cst2[:, 4:8], den4[:])
                        nc.vector.tensor_copy(cst2[:, 8:136], out_ps[:, 0:128])
                        nc.vector.tensor_copy(cst2[:, 136:264], out_ps[:, 384:512])
                        nc.sync.dma_start(out=t_cdbg2[:, :], in_=cst2[:])
                    acc = accp.tile([P, HID], f32, name="acc")
                    tmp = accp.tile([P, HID], f32, name="tmpacc")
                    nc.vector.tensor_scalar_mul(acc[:], out_ps[:, 0:HID],
                                                rec[:, 0:1])
                    for h in range(1, HEADS):
                        nc.vector.tensor_scalar_mul(
                            tmp[:], out_ps[:, h * HID:(h + 1) * HID],
                            rec[:, h:h + 1])
                        nc.vector.tensor_tensor(out=acc[:], in0=acc[:],
                                                in1=tmp[:], op=AluOp.add)
                    nc.vector.tensor_tensor(out=acc[:], in0=acc[:], in1=bbc_sb[:],
                                            op=AluOp.add)
                    nc.sync.dma_start(out=out_dram[t * P:(t + 1) * P, :],
                                      in_=acc[:])

            with nc.named_scope("phA1"):
                phase_a_l1()
            with nc.named_scope("phB1"):
                phase_b(G1, GW1, b1_sb, h_sh)
            if DEBUG:
                nc.sync.dma_start(out=t_gdbg[:, :], in_=G1[0:2 * P, :])
                nc.sync.dma_start(out=t_hdbg[:, :], in_=h_sh[:, :])
            with nc.named_scope("phAG"):
                nc.gpsimd.collective_compute(
                    "AllGather", AluOp.bypass,
                    replica_groups=[list(range(NCORES))],
                    ins=[h_sh[:].opt()], outs=[h_full[:].opt()])
            with nc.named_scope("phA2"):
                phase_a_l2()
            with nc.named_scope("phB2"):
                phase_b(G2, GW2, b2_sb, t_out[:, :])

    nc.compile()
    return nc


def _get_nc(s):
    key = tuple(sorted(s.items()))
    if key not in _CACHE:
        _CACHE[key] = _build(s)
    return _CACHE[key]


def kernel(**inputs):
    from concourse import bass_utils

    x = np.asarray(inputs["x"], dtype=np.float32)
    edge_index = np.asarray(inputs["edge_index"])
    args = (x, edge_index,
            np.asarray(inputs["W1"], np.float32),
            np.asarray(inputs["att_src1"], np.float32),
            np.asarray(inputs["att_dst1"], np.float32),
            np.asarray(inputs["b1"], np.float32),
            np.asarray(inputs["W2"], np.float32),
            np.asarray(inputs["att_src2"], np.float32),
            np.asarray(inputs["att_dst2"], np.float32),
            np.asarray(inputs["b2"], np.float32))
    shapes, shared, percore = _host_prep(*args)
    nc = _get_nc(shapes)

    in_maps = []
    for i in range(NCORES):
        m = {"xT": shared["xT"], "W1cat": shared["W1cat"],
             "W2cat": shared["W2cat"], "IOTA": shared["IOTA"],
             "IDENT": shared["IDENT"], "B1bc": shared["B1bc"],
             "B2bc": shared["B2bc"], "SRC": percore[i]["SRC"],
             "TILEIDX": percore[i]["TILEIDX"], "DSTL": percore[i]["DSTL"]}
        in_maps.append(m)

    res = bass_utils.run_bass_kernel_spmd(nc, in_maps,
                                          core_ids=list(range(NCORES)))
    out = np.concatenate([res.results[i]["out"] for i in range(NCORES)],
                         axis=0)[:shapes["N"]]
    return np.ascontiguousarray(out, dtype=np.float32)
